# revision 46
# baseline (speedup 1.0000x reference)
"""DemodulatedLinear Trainium2 kernel (v3).

Reference computation (B=1024, IN=512, OUT=512, MOD=256):
    scales = modulations @ mod_w.T + mod_b                    # [B, IN]
    w1     = weight[None] * scales[:, None, :]                # [B, OUT, IN]
    w2     = w1 * rsqrt(sum(w1^2, axis=-2) + eps)             # col L2 renorm
    out    = einsum("bi,boi->bo", x, w2) + bias               # [B, OUT]

Since sum_o w1[b,o,i]^2 = scales[b,i]^2 * c2[i] with c2[i] = sum_o w[o,i]^2,
fold sqc = sqrt(c2) into the params ON HOST:
    modw' = mod_w * sqc[:,None],  modb' = mod_b * sqc,  wT' = w.T / sqc[:,None]
so that with s' = modulations @ modw'.T + modb'  (= sqc * scales):
    y   = x * s' * rsqrt(s'^2 + eps)                          # [B, IN]
    out = y @ wT' + bias                                      # [B, OUT]
No colnorm work on device at all.

Sharding: data-parallel over batch, 8 cores x 128 rows. Params replicated.
Layout: i (IN) on partitions in 4 chunks of 128, b on free dim (so both
matmuls contract on partitions and no on-device transpose is needed).

Precision: mm1 is exact fp32 (the oracle's scales sign must be matched:
rsqrt makes y ~ x*sign(s)/sqc, so an s-relative-error eps costs ~sqrt(eps)
in output rel-err via sign flips near s=0). mm2 and x run in bf16 (plain
linear contributions, ~2.5e-3 rel-err total, fine for the 2e-2 gate).

Per chunk j (5 ops; GPSIMD cannot read PSUM, so both PSUM readers are
ACT/DVE and the GP op is SBUF-only):
    t2 = ACT Square(ps, bias=modb')      # s'^2       (reads PSUM)
    xs = DVE (ps + modb') * xT           # scalar_tensor_tensor (reads PSUM),
                                         #   independent of t2/r -> runs early
    r  = ACT Abs_reciprocal_sqrt(t2, bias=eps)
    y  = GP  xs * r -> bf16
    mm2: po += y^T @ wT'_j (bf16)

mm1's pack is split in two DMAs (chunks 0-1 + mods/modb first, chunks 2-3
second) so ps_0/ps_1 land ~1.3us before the full modw' transfer finishes.
All input DMA rides ONE HWDGE ring in consumption order so each transfer
gets full HBM bandwidth. Warmup bf16 matmuls read garbage SBUF (outputs
unused) so they issue immediately at body start and lift the PE HAM clock
gate before mm1; ACT tables are prefetched with dummy activations.
"""

import numpy as np
import ml_dtypes

import concourse.bacc as bacc
import concourse.mybir as mybir
import concourse.tile as tile
from concourse.bass_utils import run_bass_kernel_spmd

N_CORES = 8
B, IN_DIM, OUT_DIM, MOD_DIM = 1024, 512, 512, 256
BS = B // N_CORES  # 128 batch rows per core
P = 128
KI = IN_DIM // P   # 4 i-chunks
KM = MOD_DIM // P  # 2 m-chunks
EPS = 1e-8

F32 = mybir.dt.float32
F32R = mybir.dt.float32r
BF16 = mybir.dt.bfloat16
AF = mybir.ActivationFunctionType
ALU = mybir.AluOpType


WARMUP_MM = 6  # dummy bf16 matmuls to lift the PE HAM clock gate during DMA
ABS_RSQRT = True  # False: Sqrt + DVE reciprocal (CoreSim lacks Abs_reciprocal_sqrt)

# pack_a1: modw' j0 k0 | mods k0  (smallest possible first transfer: the
# j0-k0 matmul -- and with it the whole PE->ACT pipeline -- starts ~0.5us
# earlier than with a single j0 pack)
CA1 = P + BS  # 256 cols
# pack_a2: modw' j0 k1 | mods k1 | modb'
CA2 = P + BS + KI  # 260 cols
# pack_b: modw' j1 slices
CB = 2 * P  # 256 cols
# pack_c: modw' j2+j3 slices
CC = 2 * 2 * P  # 512 cols


def build_nc():
    nc = bacc.Bacc(None, target_bir_lowering=False)

    pka1_d = nc.dram_tensor("packa1", [P, CA1], F32, kind="ExternalInput")
    pka2_d = nc.dram_tensor("packa2", [P, CA2], F32, kind="ExternalInput")
    pkb_d = nc.dram_tensor("packb", [P, CB], F32, kind="ExternalInput")
    pkc_d = nc.dram_tensor("packc", [P, CC], F32, kind="ExternalInput")
    xp_d = nc.dram_tensor("xpack", [P, KI * BS], BF16, kind="ExternalInput")
    wtb_d = nc.dram_tensor("wtb", [P, KI * OUT_DIM], BF16, kind="ExternalInput")
    bias_d = nc.dram_tensor("bias", [1, OUT_DIM], F32, kind="ExternalInput")
    out_d = nc.dram_tensor("out", [BS, OUT_DIM], F32, kind="ExternalOutput")

    with tile.TileContext(nc) as tc:
        with (
            tc.tile_pool(name="pool", bufs=1) as pool,
            tc.tile_pool(name="psum", bufs=1, space="PSUM") as psum,
        ):
            # ---- warmups first in program order: their GP memsets and the
            # PE matmuls have no input deps, so they start at body open and
            # lift the PE HAM clock gate before mm1's operands arrive
            if WARMUP_MM:
                wl = pool.tile([P, P], BF16, tag="warm_lhs")
                nc.gpsimd.memset(wl[:], 0.0)
                wr = pool.tile([P, OUT_DIM], BF16, tag="warm_rhs")
                nc.gpsimd.memset(wr[:], 0.0)
                wp_ps = psum.tile([P, OUT_DIM], F32, tag="warm_ps")
                for _ in range(WARMUP_MM):
                    nc.tensor.matmul(wp_ps[:], wl[:], wr[:], start=True, stop=True)

            # ---- input DMA: all big loads on the Sync HWDGE ring, in
            # consumption order, so each transfer gets full HBM bandwidth
            pka1 = pool.tile([P, CA1], F32, tag="pka1")
            nc.sync.dma_start(out=pka1[:], in_=pka1_d[:])
            pka2 = pool.tile([P, CA2], F32, tag="pka2")
            nc.sync.dma_start(out=pka2[:], in_=pka2_d[:])
            pkb = pool.tile([P, CB], F32, tag="pkb")
            nc.sync.dma_start(out=pkb[:], in_=pkb_d[:])
            pkc = pool.tile([P, CC], F32, tag="pkc")
            nc.sync.dma_start(out=pkc[:], in_=pkc_d[:])
            xp = pool.tile([P, KI * BS], BF16, tag="xp")
            nc.sync.dma_start(out=xp[:], in_=xp_d[:])
            wtb = pool.tile([P, KI * OUT_DIM], BF16, tag="wtb")
            nc.sync.dma_start(out=wtb[:], in_=wtb_d[:])

            # lhsT slice for mm1 chunk (k, j)
            def modw_sl(k, j):
                if j == 0:
                    return (pka1 if k == 0 else pka2)[:, 0:P]
                if j == 1:
                    return pkb[:, k * P:(k + 1) * P]
                return pkc[:, (j - 2) * 2 * P + k * P:(j - 2) * 2 * P + (k + 1) * P]

            mods_sb = [
                pka1[:, P:P + BS],
                pka2[:, P:P + BS],
            ]
            modb_sb = pka2[:, P + BS:P + BS + KI]
            xT_sb = [xp[:, j * BS:(j + 1) * BS] for j in range(KI)]
            bias_sb = pool.tile([1, OUT_DIM], F32R, tag="bias")
            nc.gpsimd.dma_start(out=bias_sb[:], in_=bias_d[:].bitcast(F32R))

            # ---- constants (bias matmul runs in f32r: ones are exact in
            # TF32, only the small additive bias term is rounded)
            ones_f = pool.tile([1, P], F32, tag="ones_f")
            nc.vector.memset(ones_f[:], 1.0)
            ones_sb = pool.tile([1, P], F32R, tag="ones")
            nc.vector.tensor_scalar_mul(ones_sb[:], ones_f[:], 1.0)
            eps_sb = pool.tile([P, 1], F32, tag="eps")
            nc.vector.memset(eps_sb[:], EPS)
            warm_act = pool.tile([P, 1], F32, tag="warm_act")
            nc.scalar.activation(
                warm_act[:], eps_sb[:],
                AF.Abs_reciprocal_sqrt if ABS_RSQRT else AF.Sqrt,
            )
            nc.scalar.activation(warm_act[:], eps_sb[:], AF.Square)

            # ---- mm1 (fp32 exact; j-outer so ps_j completes early, in order)
            ps_sb = []
            for j in range(KI):
                ps = psum.tile([P, BS], F32, tag=f"ps_s{j}")
                for k in range(KM):
                    nc.tensor.matmul(
                        ps[:],
                        modw_sl(k, j),
                        mods_sb[k][:],
                        start=(k == 0),
                        stop=(k == KM - 1),
                    )
                ps_sb.append(ps)

            # ---- mm2 accumulates into TWO half-width banks so the first
            # output copy+DMA can start while the second half's last matmul
            # still runs; bias matmuls open both groups early
            H = OUT_DIM // 2
            po_a = psum.tile([P, H], F32, tag="po_a")
            po_b = psum.tile([P, H], F32, tag="po_b")
            nc.tensor.matmul(
                po_a[:], ones_sb[:], bias_sb[:, 0:H], start=True, stop=False
            )
            nc.tensor.matmul(
                po_b[:], ones_sb[:], bias_sb[:, H:OUT_DIM], start=True, stop=False
            )

            # ---- demod chain per chunk, then its mm2 contribution (bf16)
            for j in range(KI):
                mb = modb_sb[:, j:j + 1]
                t2 = pool.tile([P, BS], F32, tag=f"t{j}")
                nc.scalar.activation(t2[:], ps_sb[j][:], AF.Square, bias=mb)
                xs = pool.tile([P, BS], F32, tag=f"xs{j}")
                nc.vector.scalar_tensor_tensor(
                    xs[:], ps_sb[j][:], mb, xT_sb[j][:], ALU.add, ALU.mult
                )
                r = pool.tile([P, BS], F32, tag=f"r{j}")
                if ABS_RSQRT:
                    nc.scalar.activation(
                        r[:], t2[:], AF.Abs_reciprocal_sqrt, bias=eps_sb[:]
                    )
                else:
                    u = pool.tile([P, BS], F32, tag=f"u{j}")
                    nc.scalar.activation(u[:], t2[:], AF.Sqrt, bias=eps_sb[:])
                    nc.vector.reciprocal_approx_fast(r[:], u[:])
                y = pool.tile([P, BS], BF16, tag=f"y{j}")
                nc.vector.tensor_mul(y[:], xs[:], r[:])
                nc.tensor.matmul(
                    po_a[:], y[:], wtb[:, j * OUT_DIM:j * OUT_DIM + H],
                    start=False, stop=(j == KI - 1),
                )
                nc.tensor.matmul(
                    po_b[:], y[:], wtb[:, j * OUT_DIM + H:(j + 1) * OUT_DIM],
                    start=False, stop=(j == KI - 1),
                )

            # ---- store halves as each half-bank closes
            ob0 = pool.tile([P, H], F32, tag="ob0")
            nc.scalar.activation(ob0[:], po_a[:], AF.Copy)
            nc.sync.dma_start(out=out_d[:, 0:H], in_=ob0[:])
            ob1 = pool.tile([P, H], F32, tag="ob1")
            nc.vector.tensor_copy(ob1[:], po_b[:])
            nc.scalar.dma_start(out=out_d[:, H:OUT_DIM], in_=ob1[:])

    nc.finalize()
    return nc


def prep_in_maps(modulations, x, weight, bias, mod_w, mod_b):
    modulations = np.asarray(modulations, dtype=np.float32)
    x = np.asarray(x, dtype=np.float32)
    weight = np.asarray(weight, dtype=np.float32)
    bias = np.asarray(bias, dtype=np.float32)
    mod_w = np.asarray(mod_w, dtype=np.float32)
    mod_b = np.asarray(mod_b, dtype=np.float32)

    # fold sqrt(colnorm2) into the params (host-side, fp64 for the norm)
    c2 = np.square(weight.astype(np.float64)).sum(axis=0)
    sqc = np.sqrt(c2).astype(np.float32)                # [IN]
    modw_f = (mod_w * sqc[:, None]).astype(np.float32)  # [IN, MOD]
    modb_f = (mod_b * sqc).astype(np.float32)           # [IN]
    wt_f = (weight.T / sqc[:, None]).astype(ml_dtypes.bfloat16)  # [IN, OUT]

    modwT = modw_f.T.reshape(KM, P, KI, P)              # [k, p, j, i']
    bias_row = np.ascontiguousarray(bias.reshape(1, OUT_DIM))
    wtb = np.ascontiguousarray(
        wt_f.reshape(KI, P, OUT_DIM).transpose(1, 0, 2).reshape(P, KI * OUT_DIM)
    )
    pka1 = np.empty((P, CA1), np.float32)
    pka2 = np.empty((P, CA2), np.float32)
    pkb = np.empty((P, CB), np.float32)
    pkc = np.empty((P, CC), np.float32)
    pka1[:, 0:P] = modwT[0, :, 0]
    pka2[:, 0:P] = modwT[1, :, 0]
    for k in range(KM):
        pkb[:, k * P:(k + 1) * P] = modwT[k, :, 1]
        for j in range(2):
            pkc[:, j * 2 * P + k * P:j * 2 * P + (k + 1) * P] = modwT[k, :, 2 + j]
    pka2[:, P + BS:P + BS + KI] = modb_f.reshape(KI, P).T
    in_maps = []
    for c in range(N_CORES):
        sl = slice(c * BS, (c + 1) * BS)
        pa1 = pka1.copy()
        pa2 = pka2.copy()
        modsT = modulations[sl].T.reshape(KM, P, BS)
        pa1[:, P:P + BS] = modsT[0]
        pa2[:, P:P + BS] = modsT[1]
        xT = x[sl].T.reshape(KI, P, BS)
        xpack = np.ascontiguousarray(
            xT.transpose(1, 0, 2).reshape(P, KI * BS)
        ).astype(ml_dtypes.bfloat16)
        in_maps.append({
            "packa1": pa1,
            "packa2": pa2,
            "packb": pkb,
            "packc": pkc,
            "xpack": xpack,
            "wtb": wtb,
            "bias": bias_row,
        })
    return in_maps


_NC_CACHE = []


def _get_nc():
    if not _NC_CACHE:
        _NC_CACHE.append(build_nc())
    return _NC_CACHE[0]


def run(in_maps, **kwargs):
    nc = _get_nc()
    return run_bass_kernel_spmd(nc, in_maps, list(range(N_CORES)), **kwargs)


def kernel(modulations, x, weight, bias, mod_w, mod_b):
    in_maps = prep_in_maps(modulations, x, weight, bias, mod_w, mod_b)
    res = run(in_maps)
    return np.concatenate([res.results[c]["out"] for c in range(N_CORES)], axis=0)


# revision 49
# speedup vs baseline: 1.1381x; 1.1381x over previous
"""DemodulatedLinear Trainium2 kernel (v3).

Reference computation (B=1024, IN=512, OUT=512, MOD=256):
    scales = modulations @ mod_w.T + mod_b                    # [B, IN]
    w1     = weight[None] * scales[:, None, :]                # [B, OUT, IN]
    w2     = w1 * rsqrt(sum(w1^2, axis=-2) + eps)             # col L2 renorm
    out    = einsum("bi,boi->bo", x, w2) + bias               # [B, OUT]

Since sum_o w1[b,o,i]^2 = scales[b,i]^2 * c2[i] with c2[i] = sum_o w[o,i]^2,
fold sqc = sqrt(c2) into the params ON HOST:
    modw' = mod_w * sqc[:,None],  modb' = mod_b * sqc,  wT' = w.T / sqc[:,None]
so that with s' = modulations @ modw'.T + modb'  (= sqc * scales):
    y   = x * s' * rsqrt(s'^2 + eps)                          # [B, IN]
    out = y @ wT' + bias                                      # [B, OUT]
No colnorm work on device at all.

Sharding: data-parallel over batch, 8 cores x 128 rows. Params replicated.
Layout: i (IN) on partitions in 4 chunks of 128, b on free dim (so both
matmuls contract on partitions and no on-device transpose is needed).

Precision: mm1 is exact fp32 (the oracle's scales sign must be matched:
rsqrt makes y ~ x*sign(s)/sqc, so an s-relative-error eps costs ~sqrt(eps)
in output rel-err via sign flips near s=0). mm2 and x run in bf16 (plain
linear contributions, ~2.5e-3 rel-err total, fine for the 2e-2 gate).

Per chunk j (5 ops; GPSIMD cannot read PSUM, so both PSUM readers are
ACT/DVE and the GP op is SBUF-only):
    t2 = ACT Square(ps, bias=modb')      # s'^2       (reads PSUM)
    xs = DVE (ps + modb') * xT           # scalar_tensor_tensor (reads PSUM),
                                         #   independent of t2/r -> runs early
    r  = ACT Abs_reciprocal_sqrt(t2, bias=eps)
    y  = GP  xs * r -> bf16
    mm2: po += y^T @ wT'_j (bf16)

mm1's pack is split in two DMAs (chunks 0-1 + mods/modb first, chunks 2-3
second) so ps_0/ps_1 land ~1.3us before the full modw' transfer finishes.
All input DMA rides ONE HWDGE ring in consumption order so each transfer
gets full HBM bandwidth. Warmup bf16 matmuls read garbage SBUF (outputs
unused) so they issue immediately at body start and lift the PE HAM clock
gate before mm1; ACT tables are prefetched with dummy activations.
"""

import numpy as np
import ml_dtypes

import concourse.bacc as bacc
import concourse.mybir as mybir
import concourse.tile as tile
from concourse.bass_utils import run_bass_kernel_spmd

N_CORES = 8
B, IN_DIM, OUT_DIM, MOD_DIM = 1024, 512, 512, 256
BS = B // N_CORES  # 128 batch rows per core
P = 128
KI = IN_DIM // P   # 4 i-chunks
KM = MOD_DIM // P  # 2 m-chunks
EPS = 1e-8

F32 = mybir.dt.float32
F32R = mybir.dt.float32r
BF16 = mybir.dt.bfloat16
AF = mybir.ActivationFunctionType
ALU = mybir.AluOpType


WARMUP_MM = 5  # dummy bf16 matmuls to lift the PE HAM clock gate during DMA
# (5 x ~430ns cold from ~7.9us ends ~9.97us, right at pka's completion
# semaphore ~9.9us -- a 6th warmup would block mm1's first matmul)
ABS_RSQRT = True  # False: Sqrt + DVE reciprocal (CoreSim lacks Abs_reciprocal_sqrt)

# pack_a: modw' j0 slices (2 k-chunks) | mods (2 k-chunks) | modb'
CA = 2 * P + KM * BS + KI  # 516 cols
# pack_b: modw' j1 slices
CB = 2 * P  # 256 cols
# pack_c: modw' j2+j3 slices
CC = 2 * 2 * P  # 512 cols


def build_nc():
    nc = bacc.Bacc(None, target_bir_lowering=False)

    pka_d = nc.dram_tensor("packa", [P, CA], F32, kind="ExternalInput")
    pkb_d = nc.dram_tensor("packb", [P, CB], F32, kind="ExternalInput")
    pkc_d = nc.dram_tensor("packc", [P, CC], F32, kind="ExternalInput")
    xp_d = nc.dram_tensor("xpack", [P, KI * BS], BF16, kind="ExternalInput")
    wtb_d = nc.dram_tensor("wtb", [P, KI * OUT_DIM], BF16, kind="ExternalInput")
    bias_d = nc.dram_tensor("bias", [1, OUT_DIM], F32, kind="ExternalInput")
    out_d = nc.dram_tensor("out", [BS, OUT_DIM], F32, kind="ExternalOutput")

    with tile.TileContext(nc) as tc:
        with (
            tc.tile_pool(name="pool", bufs=1) as pool,
            tc.tile_pool(name="psum", bufs=1, space="PSUM") as psum,
        ):
            # ---- warmups first in program order: their GP memsets and the
            # PE matmuls have no input deps, so they start at body open and
            # lift the PE HAM clock gate before mm1's operands arrive
            if WARMUP_MM:
                wl = pool.tile([P, P], BF16, tag="warm_lhs")
                nc.gpsimd.memset(wl[:], 0.0)
                wr = pool.tile([P, OUT_DIM], BF16, tag="warm_rhs")
                nc.gpsimd.memset(wr[:], 0.0)
                wp_ps = psum.tile([P, OUT_DIM], F32, tag="warm_ps")
                for _ in range(WARMUP_MM):
                    nc.tensor.matmul(wp_ps[:], wl[:], wr[:], start=True, stop=True)

            # ---- input DMA: all big loads on the Sync HWDGE ring, in
            # consumption order, so each transfer gets full HBM bandwidth
            pka = pool.tile([P, CA], F32, tag="pka")
            nc.sync.dma_start(out=pka[:], in_=pka_d[:])
            pkb = pool.tile([P, CB], F32, tag="pkb")
            nc.sync.dma_start(out=pkb[:], in_=pkb_d[:])
            pkc = pool.tile([P, CC], F32, tag="pkc")
            nc.sync.dma_start(out=pkc[:], in_=pkc_d[:])
            xp = pool.tile([P, KI * BS], BF16, tag="xp")
            nc.sync.dma_start(out=xp[:], in_=xp_d[:])
            wtb = pool.tile([P, KI * OUT_DIM], BF16, tag="wtb")
            nc.sync.dma_start(out=wtb[:], in_=wtb_d[:])

            # lhsT slice for mm1 chunk (k, j)
            def modw_sl(k, j):
                if j == 0:
                    return pka[:, k * P:(k + 1) * P]
                if j == 1:
                    return pkb[:, k * P:(k + 1) * P]
                return pkc[:, (j - 2) * 2 * P + k * P:(j - 2) * 2 * P + (k + 1) * P]

            mods_sb = [pka[:, 2 * P + k * BS:2 * P + (k + 1) * BS] for k in range(KM)]
            modb_sb = pka[:, 2 * P + KM * BS:2 * P + KM * BS + KI]
            xT_sb = [xp[:, j * BS:(j + 1) * BS] for j in range(KI)]
            bias_sb = pool.tile([1, OUT_DIM], F32R, tag="bias")
            nc.gpsimd.dma_start(out=bias_sb[:], in_=bias_d[:].bitcast(F32R))

            # ---- constants (bias matmul runs in f32r: ones are exact in
            # TF32, only the small additive bias term is rounded)
            ones_f = pool.tile([1, P], F32, tag="ones_f")
            nc.vector.memset(ones_f[:], 1.0)
            ones_sb = pool.tile([1, P], F32R, tag="ones")
            nc.vector.tensor_scalar_mul(ones_sb[:], ones_f[:], 1.0)
            eps_sb = pool.tile([P, 1], F32, tag="eps")
            nc.vector.memset(eps_sb[:], EPS)
            warm_act = pool.tile([P, 1], F32, tag="warm_act")
            nc.scalar.activation(
                warm_act[:], eps_sb[:],
                AF.Abs_reciprocal_sqrt if ABS_RSQRT else AF.Sqrt,
            )
            nc.scalar.activation(warm_act[:], eps_sb[:], AF.Square)

            # ---- mm1 (fp32 exact; j-outer so ps_j completes early, in order)
            ps_sb = []
            for j in range(KI):
                ps = psum.tile([P, BS], F32, tag=f"ps_s{j}")
                for k in range(KM):
                    nc.tensor.matmul(
                        ps[:],
                        modw_sl(k, j),
                        mods_sb[k][:],
                        start=(k == 0),
                        stop=(k == KM - 1),
                    )
                ps_sb.append(ps)

            # ---- mm2 accumulates into TWO half-width banks so the first
            # output copy+DMA can start while the second half's last matmul
            # still runs; bias matmuls open both groups early
            H = OUT_DIM // 2
            po_a = psum.tile([P, H], F32, tag="po_a")
            po_b = psum.tile([P, H], F32, tag="po_b")
            nc.tensor.matmul(
                po_a[:], ones_sb[:], bias_sb[:, 0:H], start=True, stop=False
            )
            nc.tensor.matmul(
                po_b[:], ones_sb[:], bias_sb[:, H:OUT_DIM], start=True, stop=False
            )

            # ---- demod chain per chunk, then its mm2 contribution (bf16)
            for j in range(KI):
                mb = modb_sb[:, j:j + 1]
                t2 = pool.tile([P, BS], F32, tag=f"t{j}")
                nc.scalar.activation(t2[:], ps_sb[j][:], AF.Square, bias=mb)
                xs = pool.tile([P, BS], F32, tag=f"xs{j}")
                nc.vector.scalar_tensor_tensor(
                    xs[:], ps_sb[j][:], mb, xT_sb[j][:], ALU.add, ALU.mult
                )
                r = pool.tile([P, BS], F32, tag=f"r{j}")
                if ABS_RSQRT:
                    nc.scalar.activation(
                        r[:], t2[:], AF.Abs_reciprocal_sqrt, bias=eps_sb[:]
                    )
                else:
                    u = pool.tile([P, BS], F32, tag=f"u{j}")
                    nc.scalar.activation(u[:], t2[:], AF.Sqrt, bias=eps_sb[:])
                    nc.vector.reciprocal_approx_fast(r[:], u[:])
                y = pool.tile([P, BS], BF16, tag=f"y{j}")
                nc.vector.tensor_mul(y[:], xs[:], r[:])
                nc.tensor.matmul(
                    po_a[:], y[:], wtb[:, j * OUT_DIM:j * OUT_DIM + H],
                    start=False, stop=(j == KI - 1),
                )
                nc.tensor.matmul(
                    po_b[:], y[:], wtb[:, j * OUT_DIM + H:(j + 1) * OUT_DIM],
                    start=False, stop=(j == KI - 1),
                )

            # ---- store halves as each half-bank closes
            ob0 = pool.tile([P, H], F32, tag="ob0")
            nc.scalar.activation(ob0[:], po_a[:], AF.Copy)
            nc.sync.dma_start(out=out_d[:, 0:H], in_=ob0[:])
            ob1 = pool.tile([P, H], F32, tag="ob1")
            nc.vector.tensor_copy(ob1[:], po_b[:])
            nc.scalar.dma_start(out=out_d[:, H:OUT_DIM], in_=ob1[:])

    nc.finalize()
    return nc


def prep_in_maps(modulations, x, weight, bias, mod_w, mod_b):
    modulations = np.asarray(modulations, dtype=np.float32)
    x = np.asarray(x, dtype=np.float32)
    weight = np.asarray(weight, dtype=np.float32)
    bias = np.asarray(bias, dtype=np.float32)
    mod_w = np.asarray(mod_w, dtype=np.float32)
    mod_b = np.asarray(mod_b, dtype=np.float32)

    # fold sqrt(colnorm2) into the params (host-side, fp64 for the norm)
    c2 = np.square(weight.astype(np.float64)).sum(axis=0)
    sqc = np.sqrt(c2).astype(np.float32)                # [IN]
    modw_f = (mod_w * sqc[:, None]).astype(np.float32)  # [IN, MOD]
    modb_f = (mod_b * sqc).astype(np.float32)           # [IN]
    wt_f = (weight.T / sqc[:, None]).astype(ml_dtypes.bfloat16)  # [IN, OUT]

    modwT = modw_f.T.reshape(KM, P, KI, P)              # [k, p, j, i']
    bias_row = np.ascontiguousarray(bias.reshape(1, OUT_DIM))
    wtb = np.ascontiguousarray(
        wt_f.reshape(KI, P, OUT_DIM).transpose(1, 0, 2).reshape(P, KI * OUT_DIM)
    )
    pka = np.empty((P, CA), np.float32)
    pkb = np.empty((P, CB), np.float32)
    pkc = np.empty((P, CC), np.float32)
    for k in range(KM):
        pka[:, k * P:(k + 1) * P] = modwT[k, :, 0]
        pkb[:, k * P:(k + 1) * P] = modwT[k, :, 1]
        for j in range(2):
            pkc[:, j * 2 * P + k * P:j * 2 * P + (k + 1) * P] = modwT[k, :, 2 + j]
    pka[:, 2 * P + KM * BS:2 * P + KM * BS + KI] = modb_f.reshape(KI, P).T
    in_maps = []
    for c in range(N_CORES):
        sl = slice(c * BS, (c + 1) * BS)
        pa = pka.copy()
        modsT = modulations[sl].T.reshape(KM, P, BS)
        for k in range(KM):
            pa[:, 2 * P + k * BS:2 * P + (k + 1) * BS] = modsT[k]
        xT = x[sl].T.reshape(KI, P, BS)
        xpack = np.ascontiguousarray(
            xT.transpose(1, 0, 2).reshape(P, KI * BS)
        ).astype(ml_dtypes.bfloat16)
        in_maps.append({
            "packa": pa,
            "packb": pkb,
            "packc": pkc,
            "xpack": xpack,
            "wtb": wtb,
            "bias": bias_row,
        })
    return in_maps


_NC_CACHE = []


def _get_nc():
    if not _NC_CACHE:
        _NC_CACHE.append(build_nc())
    return _NC_CACHE[0]


def run(in_maps, **kwargs):
    nc = _get_nc()
    return run_bass_kernel_spmd(nc, in_maps, list(range(N_CORES)), **kwargs)


def kernel(modulations, x, weight, bias, mod_w, mod_b):
    in_maps = prep_in_maps(modulations, x, weight, bias, mod_w, mod_b)
    res = run(in_maps)
    return np.concatenate([res.results[c]["out"] for c in range(N_CORES)], axis=0)
